# revision 7
# baseline (speedup 1.0000x reference)
# CATS-SwiGLU decode kernel for TRN2 (8 NeuronCores, SPMD tensor-parallel).
#
# Reference computation (decode path, B=S=1):
#   x1    = silu(x @ Wgatet)                  [1,1,dff]
#   flags = |x1| > threshold
#   z     = where(flags, (x @ Wup.T) * x1, 0) [1,1,dff]
#   out   = z @ Wdownt                        [1,1,d]
#
# Sharding: d_ff (11008) split across 8 cores (1376 rows each). Each core
# computes its z slice and a full-width partial down-projection; the host
# sums the 8 partials (the all-reduce of the TP hint, done on host).
#
# The kernel is HBM-bound: all weight bytes stream exactly once. Weights are
# cast to fp16 on the host (~0.05% per-element rounding, far inside the 2e-2
# gate), halving HBM traffic vs fp32. All three GEMVs run on the TensorEngine
# as M=1 matmuls — the x / z vector is the stationary operand (1-column
# LdWeights, ~1ns) and the weight tiles stream as the moving operand, so the
# PE consumes tiles at ~1 column/cycle and stays well under the DMA rate.
# The DVE/Act engines only handle the tiny [128,11] silu/threshold/mask chain
# and PSUM drains. Gate/up accumulate into PSUM rows [1,1376]; those rows are
# transposed into [128,11] via K=1 matmuls against a ones scalar so the z
# chunks land partition-major, ready to serve as down-projection stationaries.
import sys

for _p in ("/opt/trn_rl_repo",):
    if _p not in sys.path:
        sys.path.insert(0, _p)

import numpy as np

import concourse.bass as bass
import concourse.tile as tile
from concourse import bacc, mybir
from concourse.bass_utils import run_bass_kernel_spmd

D = 4096
FF = 11008
NCORES = 8
FSH = FF // NCORES            # 1376 rows of d_ff per core
NCH = (FSH + 127) // 128      # 11 f-chunks of <=128
LAST = FSH - 128 * (NCH - 1)  # 96 rows in the last chunk
NDC = D // 128                # 32 d-chunks
G = 4                         # d-chunks per gate/up DMA tile
NT = NDC // G                 # 8 DMA tiles per gate/up matrix
F32 = mybir.dt.float32
F16 = mybir.dt.float16
ACT = mybir.ActivationFunctionType

_CACHE = {}


def _bcast(ap, parts):
    """Replicate a 1-D AP across `parts` partitions (0-stride partition dim)."""
    return bass.AP(tensor=ap.tensor, offset=ap.offset, ap=[[0, parts]] + list(ap.ap))


def _build_nc():
    nc = bacc.Bacc("TRN2", target_bir_lowering=False, debug=False)

    x_d = nc.dram_tensor("x", [128, NDC], F16, kind="ExternalInput")
    wg_d = nc.dram_tensor("wg", [NT, 128, G * FSH], F16, kind="ExternalInput")
    wu_d = nc.dram_tensor("wu", [NT, 128, G * FSH], F16, kind="ExternalInput")
    wd_d = nc.dram_tensor("wd", [FSH, D], F16, kind="ExternalInput")
    thr_d = nc.dram_tensor("thr", [1], F32, kind="ExternalInput")
    out_d = nc.dram_tensor("out", [1, D], F32, kind="ExternalOutput")

    NSPL = ((0, 512), (512, 1024), (1024, FSH))

    with tile.TileContext(nc) as tc:
        with (
            tc.tile_pool(name="const", bufs=1) as const_pool,
            tc.tile_pool(name="wpool", bufs=6) as wpool,
            tc.tile_pool(name="dpool", bufs=NCH) as dpool,
            tc.tile_pool(name="acts", bufs=1) as acts,
        ):
            # constants on the scalar (qAct) ring so the weight stream on
            # the sync (qSP) ring starts at t=0
            x_sb = const_pool.tile([128, NDC], F16)
            nc.scalar.dma_start(out=x_sb[:], in_=x_d.ap())
            thr_sb = const_pool.tile([128, 1], F32)
            nc.scalar.dma_start(out=thr_sb[:], in_=_bcast(thr_d.ap(), 128))
            one_sb = const_pool.tile([1, 1], F16)
            nc.vector.memset(one_sb[:], 1.0)

            # warm the silu_and_others ACT table while the DMA stream runs
            warm = acts.tile([1, 1], F32)
            nc.scalar.activation(warm[:], thr_sb[0:1, :], ACT.Silu)
            nc.scalar.activation(warm[:], thr_sb[0:1, :], ACT.Abs)

            x1row_sb = acts.tile([1, FSH], F16)
            urow_sb = acts.tile([1, FSH], F16)
            x1s = acts.tile([128, NCH], F32)
            absx = acts.tile([128, NCH], F32)
            mask = acts.tile([128, NCH], F32)
            ztmp = acts.tile([128, NCH], F32)
            zm_sb = acts.tile([128, NCH], F16)
            out_sb = acts.tile([1, D], F32)

            with tc.tile_pool(name="psA", bufs=1, space="PSUM") as psA:
                x1row = psA.tile([1, FSH], F32)
                urow = psA.tile([1, FSH], F32)
                x1tr = psA.tile([128, NCH], F32)
                utr = psA.tile([128, NCH], F32)
                nc.vector.memset(x1tr[:], 0.0)
                nc.vector.memset(utr[:], 0.0)

                def stream_tile(wdram, t, accrow):
                    # alternate DMA rings: two idle sequencers generate
                    # descriptors in parallel, halving inter-DMA bubbles
                    ring = nc.sync if t % 2 == 0 else nc.gpsimd
                    wt = wpool.tile([128, G * FSH], F16, tag="w", name="wt")
                    ring.dma_start(out=wt[:], in_=wdram.ap()[t])
                    for g in range(G):
                        c = G * t + g
                        for n0, n1 in NSPL:
                            nc.tensor.matmul(
                                out=accrow[0:1, n0:n1],
                                lhsT=x_sb[:, c : c + 1],
                                rhs=wt[:, g * FSH + n0 : g * FSH + n1],
                                start=(c == 0),
                                stop=(c == NDC - 1),
                            )

                def transpose_row(row_sb, dst):
                    # [1, FSH] row -> [128, NCH] partition-major via K=1 matmuls
                    for c in range(NCH):
                        pc = 128 if c < NCH - 1 else LAST
                        nc.tensor.matmul(
                            out=dst[:pc, c : c + 1],
                            lhsT=row_sb[0:1, c * 128 : c * 128 + pc],
                            rhs=one_sb[:],
                            start=True,
                            stop=True,
                        )

                for t in range(NT):
                    stream_tile(wg_d, t, x1row)
                stream_tile(wu_d, 0, urow)
                stream_tile(wu_d, 1, urow)
                # x1 post-processing overlaps the up stream
                nc.scalar.copy(x1row_sb[:], x1row[:])
                transpose_row(x1row_sb, x1tr)
                nc.scalar.activation(x1s[:], x1tr[:], ACT.Silu)
                nc.scalar.activation(absx[:], x1s[:], ACT.Abs)
                nc.vector.tensor_scalar(
                    out=mask[:],
                    in0=absx[:],
                    scalar1=thr_sb[:],
                    scalar2=None,
                    op0=mybir.AluOpType.is_gt,
                )
                for t in range(2, NT):
                    stream_tile(wu_d, t, urow)
                # split the PSUM->SBUF drain across Act+DVE: it gates the
                # down-projection start, so halve its latency
                HF = (FSH // 2) // 512 * 512  # 512-aligned split point
                nc.scalar.copy(urow_sb[0:1, :HF], urow[0:1, :HF])
                nc.vector.tensor_copy(urow_sb[0:1, HF:], urow[0:1, HF:])
                transpose_row(urow_sb, utr)
                nc.vector.tensor_mul(ztmp[:], utr[:], x1s[:])
                nc.vector.tensor_mul(zm_sb[:], ztmp[:], mask[:])

            with tc.tile_pool(name="psB", bufs=1, space="PSUM") as psB:
                dn = psB.tile([1, D], F32)
                for c in range(NCH):
                    pc = 128 if c < NCH - 1 else LAST
                    ring = nc.sync if c % 2 == 0 else nc.gpsimd
                    dt_ = dpool.tile([128, D], F16, tag="d", name="dt_")
                    ring.dma_start(
                        out=dt_[:pc, :], in_=wd_d.ap()[c * 128 : c * 128 + pc, :]
                    )
                    for b in range(8):
                        nc.tensor.matmul(
                            out=dn[0:1, b * 512 : (b + 1) * 512],
                            lhsT=zm_sb[:pc, c : c + 1],
                            rhs=dt_[:pc, b * 512 : (b + 1) * 512],
                            start=(c == 0),
                            stop=(c == NCH - 1),
                        )
                # drain PSUM per bank as each accumulation closes; alternate
                # Act/DVE so the tail is half as long
                for b in range(8):
                    sl = slice(b * 512, (b + 1) * 512)
                    if b % 2 == 0:
                        nc.scalar.copy(out_sb[0:1, sl], dn[0:1, sl])
                    else:
                        nc.vector.tensor_copy(out_sb[0:1, sl], dn[0:1, sl])

            # gpsimd ring: its sequencer pre-generates the descriptors long
            # before the drain copies land, so the store fires immediately
            nc.gpsimd.dma_start(out=out_d.ap(), in_=out_sb[:])

    nc.compile()
    return nc


def _get_nc():
    if "nc" not in _CACHE:
        _CACHE["nc"] = _build_nc()
    return _CACHE["nc"]


def make_in_maps(x, Wup, Wgatet, Wdownt, threshold):
    """Shard full inputs into the 8 per-core input maps (fp16 weights)."""
    x16 = np.asarray(x, dtype=np.float32).reshape(D).astype(np.float16)
    xb = np.ascontiguousarray(x16.reshape(NDC, 128).T)      # [128, 32]
    thr = np.asarray(threshold, dtype=np.float32).reshape(1)
    Wg16 = np.asarray(Wgatet, dtype=np.float32).astype(np.float16)  # [D, FF]
    Wu16 = np.asarray(Wup, dtype=np.float32).astype(np.float16)     # [FF, D]
    Wd16 = np.asarray(Wdownt, dtype=np.float32).astype(np.float16)  # [FF, D]
    in_maps = []
    for i in range(NCORES):
        sl = slice(i * FSH, (i + 1) * FSH)
        wg = (
            Wg16[:, sl]
            .reshape(NT, G, 128, FSH)
            .transpose(0, 2, 1, 3)
            .reshape(NT, 128, G * FSH)
        )
        wg = np.ascontiguousarray(wg)                        # [NT, 128, G*FSH]
        wu = (
            Wu16[sl, :]
            .T.reshape(NT, G, 128, FSH)
            .transpose(0, 2, 1, 3)
            .reshape(NT, 128, G * FSH)
        )
        wu = np.ascontiguousarray(wu)                        # [NT, 128, G*FSH]
        wd = np.ascontiguousarray(Wd16[sl, :])               # [FSH, D]
        in_maps.append({"x": xb, "wg": wg, "wu": wu, "wd": wd, "thr": thr})
    return in_maps


def run_sharded(x, Wup, Wgatet, Wdownt, threshold, trace=False, tmpdir=None):
    """Run on the 8 NeuronCores; returns (full_output, BassKernelResults)."""
    nc = _get_nc()
    in_maps = make_in_maps(x, Wup, Wgatet, Wdownt, threshold)
    res = run_bass_kernel_spmd(
        nc, in_maps, list(range(NCORES)), trace=trace, tmpdir=tmpdir
    )
    # un-shard: sum the 8 partial down-projections
    acc = np.zeros(D, dtype=np.float64)
    for r in res.results:
        acc += r["out"].reshape(D).astype(np.float64)
    out = acc.astype(np.float32).reshape(1, 1, D)
    return out, res


def kernel(x, Wup, Wgatet, Wdownt, threshold):
    out, _ = run_sharded(x, Wup, Wgatet, Wdownt, threshold)
    return out


# revision 10
# speedup vs baseline: 1.3578x; 1.3578x over previous
# CATS-SwiGLU decode kernel for TRN2 (8 NeuronCores, SPMD tensor-parallel).
#
# Reference computation (decode path, B=S=1):
#   x1    = silu(x @ Wgatet)                  [1,1,dff]
#   flags = |x1| > threshold
#   z     = where(flags, (x @ Wup.T) * x1, 0) [1,1,dff]
#   out   = z @ Wdownt                        [1,1,d]
#
# Sharding: d_ff (11008) split across 8 cores (1376 rows each). Each core
# computes its z slice and a full-width partial down-projection; the host
# sums the 8 partials (the all-reduce of the TP hint, done on host).
#
# The kernel is HBM-bound: all weight bytes stream exactly once. Weights are
# cast to fp16 on the host (~0.05% per-element rounding, far inside the 2e-2
# gate), halving HBM traffic vs fp32. All three GEMVs run on the TensorEngine
# as M=1 matmuls — the x / z vector is the stationary operand (1-column
# LdWeights, ~1ns) and the weight tiles stream as the moving operand, so the
# PE consumes tiles at ~1 column/cycle and stays well under the DMA rate.
# The DVE/Act engines only handle the tiny [128,11] silu/threshold/mask chain
# and PSUM drains. Gate/up accumulate into PSUM rows [1,1376]; those rows are
# transposed into [128,11] via K=1 matmuls against a ones scalar so the z
# chunks land partition-major, ready to serve as down-projection stationaries.
import sys

for _p in ("/opt/trn_rl_repo",):
    if _p not in sys.path:
        sys.path.insert(0, _p)

import numpy as np

import concourse.bass as bass
import concourse.tile as tile
from concourse import bacc, mybir
from concourse.bass_utils import run_bass_kernel_spmd

D = 4096
FF = 11008
NCORES = 8
FSH = FF // NCORES            # 1376 rows of d_ff per core
NCH = (FSH + 127) // 128      # 11 f-chunks of <=128
LAST = FSH - 128 * (NCH - 1)  # 96 rows in the last chunk
NDC = D // 128                # 32 d-chunks
G = 4                         # d-chunks per gate/up DMA tile
NT = NDC // G                 # 8 DMA tiles per gate/up matrix
F32 = mybir.dt.float32
F16 = mybir.dt.float16
ACT = mybir.ActivationFunctionType

_CACHE = {}


def _bcast(ap, parts):
    """Replicate a 1-D AP across `parts` partitions (0-stride partition dim)."""
    return bass.AP(tensor=ap.tensor, offset=ap.offset, ap=[[0, parts]] + list(ap.ap))


def _build_nc():
    nc = bacc.Bacc("TRN2", target_bir_lowering=False, debug=False)

    x_d = nc.dram_tensor("x", [128, NDC], F16, kind="ExternalInput")
    wg_d = nc.dram_tensor("wg", [NT, 128, G * FSH], F16, kind="ExternalInput")
    wu_d = nc.dram_tensor("wu", [NT, 128, G * FSH], F16, kind="ExternalInput")
    wd_d = nc.dram_tensor("wd", [FSH, D], F16, kind="ExternalInput")
    thr_d = nc.dram_tensor("thr", [1], F32, kind="ExternalInput")
    out_d = nc.dram_tensor("out", [1, D], F32, kind="ExternalOutput")

    NSPL = ((0, 512), (512, 1024), (1024, FSH))

    with tile.TileContext(nc) as tc:
        with (
            tc.tile_pool(name="const", bufs=1) as const_pool,
            tc.tile_pool(name="wpool", bufs=6) as wpool,
            tc.tile_pool(name="dpool", bufs=NCH) as dpool,
            tc.tile_pool(name="acts", bufs=1) as acts,
        ):
            # constants on the scalar (qAct) ring so the weight stream on
            # the sync (qSP) ring starts at t=0
            x_sb = const_pool.tile([128, NDC], F16)
            nc.scalar.dma_start(out=x_sb[:], in_=x_d.ap())
            thr_sb = const_pool.tile([128, 1], F32)
            nc.scalar.dma_start(out=thr_sb[:], in_=_bcast(thr_d.ap(), 128))
            one_sb = const_pool.tile([1, 1], F16)
            nc.vector.memset(one_sb[:], 1.0)

            # warm the silu_and_others ACT table while the DMA stream runs
            warm = acts.tile([1, 1], F32)
            nc.scalar.activation(warm[:], thr_sb[0:1, :], ACT.Silu)
            nc.scalar.activation(warm[:], thr_sb[0:1, :], ACT.Abs)

            x1row_sb = acts.tile([1, FSH], F16)
            urow_sb = acts.tile([1, FSH], F16)
            x1s = acts.tile([128, NCH], F32)
            absx = acts.tile([128, NCH], F32)
            mask = acts.tile([128, NCH], F32)
            ztmp = acts.tile([128, NCH], F32)
            zm_sb = acts.tile([128, NCH], F16)
            out_sb = acts.tile([1, D], F32)

            with tc.tile_pool(name="psA", bufs=1, space="PSUM") as psA:
                x1row = psA.tile([1, FSH], F32)
                urow = psA.tile([1, FSH], F32)
                x1tr = psA.tile([128, NCH], F32)
                utr = psA.tile([128, NCH], F32)
                nc.vector.memset(x1tr[:], 0.0)
                nc.vector.memset(utr[:], 0.0)

                def stream_tile(wdram, t, accrow):
                    wt = wpool.tile([128, G * FSH], F16, tag="w", name="wt")
                    nc.sync.dma_start(out=wt[:], in_=wdram.ap()[t])
                    for g in range(G):
                        c = G * t + g
                        for n0, n1 in NSPL:
                            nc.tensor.matmul(
                                out=accrow[0:1, n0:n1],
                                lhsT=x_sb[:, c : c + 1],
                                rhs=wt[:, g * FSH + n0 : g * FSH + n1],
                                start=(c == 0),
                                stop=(c == NDC - 1),
                            )

                def transpose_row(row_sb, dst):
                    # [1, FSH] row -> [128, NCH] partition-major via K=1 matmuls
                    for c in range(NCH):
                        pc = 128 if c < NCH - 1 else LAST
                        nc.tensor.matmul(
                            out=dst[:pc, c : c + 1],
                            lhsT=row_sb[0:1, c * 128 : c * 128 + pc],
                            rhs=one_sb[:],
                            start=True,
                            stop=True,
                        )

                for t in range(NT):
                    stream_tile(wg_d, t, x1row)
                stream_tile(wu_d, 0, urow)
                stream_tile(wu_d, 1, urow)
                # x1 post-processing overlaps the up stream
                nc.scalar.copy(x1row_sb[:], x1row[:])
                transpose_row(x1row_sb, x1tr)
                nc.scalar.activation(x1s[:], x1tr[:], ACT.Silu)
                nc.scalar.activation(absx[:], x1s[:], ACT.Abs)
                nc.vector.tensor_scalar(
                    out=mask[:],
                    in0=absx[:],
                    scalar1=thr_sb[:],
                    scalar2=None,
                    op0=mybir.AluOpType.is_gt,
                )
                for t in range(2, NT):
                    stream_tile(wu_d, t, urow)
                # split the PSUM->SBUF drain across Act+DVE: it gates the
                # down-projection start, so halve its latency
                HF = (FSH // 2) // 512 * 512  # 512-aligned split point
                nc.scalar.copy(urow_sb[0:1, :HF], urow[0:1, :HF])
                nc.vector.tensor_copy(urow_sb[0:1, HF:], urow[0:1, HF:])
                transpose_row(urow_sb, utr)
                nc.vector.tensor_mul(ztmp[:], utr[:], x1s[:])
                nc.vector.tensor_mul(zm_sb[:], ztmp[:], mask[:])

            with tc.tile_pool(name="psB", bufs=1, space="PSUM") as psB:
                dn = psB.tile([1, D], F32)
                for c in range(NCH):
                    pc = 128 if c < NCH - 1 else LAST
                    dt_ = dpool.tile([128, D], F16, tag="d", name="dt_")
                    nc.sync.dma_start(
                        out=dt_[:pc, :], in_=wd_d.ap()[c * 128 : c * 128 + pc, :]
                    )
                    for b in range(8):
                        nc.tensor.matmul(
                            out=dn[0:1, b * 512 : (b + 1) * 512],
                            lhsT=zm_sb[:pc, c : c + 1],
                            rhs=dt_[:pc, b * 512 : (b + 1) * 512],
                            start=(c == 0),
                            stop=(c == NCH - 1),
                        )
                # drain PSUM per bank as each accumulation closes; alternate
                # Act/DVE so the tail is half as long
                for b in range(8):
                    sl = slice(b * 512, (b + 1) * 512)
                    if b % 2 == 0:
                        nc.scalar.copy(out_sb[0:1, sl], dn[0:1, sl])
                    else:
                        nc.vector.tensor_copy(out_sb[0:1, sl], dn[0:1, sl])

            nc.sync.dma_start(out=out_d.ap(), in_=out_sb[:])

    nc.compile()
    return nc


def _get_nc():
    if "nc" not in _CACHE:
        _CACHE["nc"] = _build_nc()
    return _CACHE["nc"]


def make_in_maps(x, Wup, Wgatet, Wdownt, threshold):
    """Shard full inputs into the 8 per-core input maps (fp16 weights)."""
    x16 = np.asarray(x, dtype=np.float32).reshape(D).astype(np.float16)
    xb = np.ascontiguousarray(x16.reshape(NDC, 128).T)      # [128, 32]
    thr = np.asarray(threshold, dtype=np.float32).reshape(1)
    Wg16 = np.asarray(Wgatet, dtype=np.float32).astype(np.float16)  # [D, FF]
    Wu16 = np.asarray(Wup, dtype=np.float32).astype(np.float16)     # [FF, D]
    Wd16 = np.asarray(Wdownt, dtype=np.float32).astype(np.float16)  # [FF, D]
    in_maps = []
    for i in range(NCORES):
        sl = slice(i * FSH, (i + 1) * FSH)
        wg = (
            Wg16[:, sl]
            .reshape(NT, G, 128, FSH)
            .transpose(0, 2, 1, 3)
            .reshape(NT, 128, G * FSH)
        )
        wg = np.ascontiguousarray(wg)                        # [NT, 128, G*FSH]
        wu = (
            Wu16[sl, :]
            .T.reshape(NT, G, 128, FSH)
            .transpose(0, 2, 1, 3)
            .reshape(NT, 128, G * FSH)
        )
        wu = np.ascontiguousarray(wu)                        # [NT, 128, G*FSH]
        wd = np.ascontiguousarray(Wd16[sl, :])               # [FSH, D]
        in_maps.append({"x": xb, "wg": wg, "wu": wu, "wd": wd, "thr": thr})
    return in_maps


def run_sharded(x, Wup, Wgatet, Wdownt, threshold, trace=False, tmpdir=None):
    """Run on the 8 NeuronCores; returns (full_output, BassKernelResults)."""
    nc = _get_nc()
    in_maps = make_in_maps(x, Wup, Wgatet, Wdownt, threshold)
    res = run_bass_kernel_spmd(
        nc, in_maps, list(range(NCORES)), trace=trace, tmpdir=tmpdir
    )
    # un-shard: sum the 8 partial down-projections
    acc = np.zeros(D, dtype=np.float64)
    for r in res.results:
        acc += r["out"].reshape(D).astype(np.float64)
    out = acc.astype(np.float32).reshape(1, 1, D)
    return out, res


def kernel(x, Wup, Wgatet, Wdownt, threshold):
    out, _ = run_sharded(x, Wup, Wgatet, Wdownt, threshold)
    return out
